# revision 13
# baseline (speedup 1.0000x reference)
"""Supervised contrastive loss on 8 trn2 NeuronCores (Bass/Tile).

Full inputs -> full output. Sharding: rows of the (sorted-by-label,
per-core rolled) embedding matrix are split 1024/core. Each core
computes its 1024x8192 block of the similarity matrix against the full
embedding set in bf16 on the TensorEngine, reduces it to a partial
loss sum; host sums the 8 partials and divides by the (host-computed)
valid pair count.

v2: host-side normalize + transpose (kills the on-device preamble:
norms, 64 PE transposes, PSUM copies), 2048-wide exp chunks (4 PSUM
banks x 2 buffers), single exp per chunk with the ln pass reading the
positives window straight from the chunk-0 dump, ln pass software-
pipelined one row tile behind the exp sweeps, diagonal ln terms
batched into one instruction, +1/T constant folded to the host.

Key algebra: with z_ij = exp(sim_ij) and ns_i = sum_{labels differ} z_ij,
  pair_loss_ij = logaddexp(sim_ij, log ns_i) - sim_ij
              = ln(z_ij + ns_i) - sim_ij
Rows are sorted by label and rolled per-core so that all positives
(same-label columns) of each 128-row tile live in one 512-wide window
inside exp chunk 0. Same-label masks are tiny and data-dependent, so
they are precomputed host-side and DMA'd in.
"""

import math
import os
import sys

import numpy as np

for _p in ("/opt/trn_rl_repo", "/root/.axon_site/_ro/trn_rl_repo"):
    if os.path.isdir(_p) and _p not in sys.path:
        sys.path.append(_p)

B = 8192
D = 128
TEMP = 0.07
SCALE = 1.0 / TEMP
N_CORES = 8
R = B // N_CORES  # rows per core
P = 128  # partitions
CH = 2048  # exp sweep chunk width (4 psum banks, 2 buffers = all 8)
EXP_S0 = math.exp(SCALE)  # z_ii for a unit-norm row


def _split_multi_waits(nc, mybir, max_waits=1):
    """Hoist excess per-instruction sync waits onto same-engine NoOps.

    This container's walrus rejects instructions carrying more than one
    sync wait ("Too many sync wait commands"); semantics are identical
    when the preceding NoOps on the same engine perform the waits.
    """
    n_new = 0
    for func in nc.m.functions:
        for block in func.blocks:
            il = block.instructions
            i = 0
            while i < len(il):
                inst = il[i]
                si = getattr(inst, "sync_info", None)
                ow = list(si.on_wait) if (si is not None and si.on_wait) else []
                if len(ow) > max_waits:
                    keep = ow[-max_waits:]
                    hoist = ow[:-max_waits]
                    nops = []
                    for w in hoist:
                        nop = mybir.InstNoOp(
                            name=f"{inst.name}-ws{len(nops)}",
                            engine=inst.engine,
                            ins=[],
                            outs=[],
                            sync_info=mybir.SyncInfo(on_wait=[w], on_update=[]),
                        )
                        nops.append(nop)
                        n_new += 1
                    inst.sync_info = mybir.SyncInfo(
                        on_wait=keep,
                        on_update=list(si.on_update) if si.on_update else [],
                    )
                    il[i:i] = nops
                    i += len(nops)
                i += 1
    return n_new


def _build_program(WIN: int, OFF: int):
    import concourse.bass as bass
    import concourse.tile as tile
    from concourse import mybir

    f32 = mybir.dt.float32
    bf16 = mybir.dt.bfloat16
    AF = mybir.ActivationFunctionType
    OP = mybir.AluOpType

    nc = bass.Bass()
    # emb arrives pre-normalized and pre-transposed: [D, B]
    d_emb = nc.dram_tensor("emb", [D, B], bf16, kind="ExternalInput")
    d_msk = nc.dram_tensor("msk", [P, (R // P) * WIN], bf16, kind="ExternalInput")
    # per-(partition, row-tile) partial losses; host does the final reduce
    d_out = nc.dram_tensor("out", [P, R // P], f32, kind="ExternalOutput")

    NRT = R // P  # 8 row tiles owned by this core
    NCH = B // CH  # 4 exp chunks
    half = (WIN - P) // 2  # window margin each side of the 128 rows
    assert OFF - half >= 0
    assert OFF + (NRT - 1) * P - half + WIN <= CH, "window exceeds chunk 0"
    assert OFF + NRT * P <= CH, "lhsT columns exceed chunk 0"

    with tile.TileContext(nc) as tc:
        with (
            tc.tile_pool(name="emb", bufs=1) as pE,
            tc.tile_pool(name="consts", bufs=1) as pC,
            tc.tile_pool(name="parts", bufs=1) as pP,
            tc.tile_pool(name="d0", bufs=2) as pD0,
            tc.tile_pool(name="dd", bufs=2) as pD,
            tc.tile_pool(name="fw", bufs=2) as pF,
            tc.tile_pool(name="sttd", bufs=2) as pStt,
            tc.tile_pool(name="ps", bufs=2, space="PSUM") as psP,
        ):
            # ---------------- load ----------------
            # chunk 0 arrives as four 512-col pieces so the first matmul
            # (rhs cols 0..512 + lhsT of tile 0) starts as early as possible
            e0p = []
            for q in range(4):
                t = pE.tile([P, 512], bf16, tag=f"e0p{q}")
                nc.sync.dma_start(out=t, in_=d_emb[:, q * 512 : (q + 1) * 512])
                e0p.append(t)
            eTc = [None]
            for c in range(1, NCH):
                t = pE.tile([P, CH], bf16, tag=f"e{c}")
                nc.sync.dma_start(out=t, in_=d_emb[:, c * CH : (c + 1) * CH])
                eTc.append(t)
            msk = pC.tile([P, NRT, WIN], bf16, tag="msk")
            nc.sync.dma_start(
                out=msk, in_=d_msk[:, :].rearrange("p (t w) -> p t w", w=WIN)
            )

            es0 = pC.tile([P, 1], f32, tag="es0")
            nc.vector.memset(es0, EXP_S0)

            # ---------------- PE warm-up ----------------
            # HAM clock-gates the PE to 1.2GHz until it sees ~3.4us of
            # sustained busy. Burn dummy matmuls on a scratch tile during
            # the DMA wait so the main loop enters at 2.4GHz.
            wm = pC.tile([P, 512], bf16, tag="wm")
            nc.gpsimd.memset(wm, 0.0)
            gw = psP.tile([P, CH], f32, tag="g")
            for _ in range(8):
                nc.tensor.matmul(
                    gw[:, :512],
                    lhsT=wm[:, :P],
                    rhs=wm,
                    start=True,
                    stop=True,
                )

            # per-row-tile accumulators
            tT = pP.tile([P, NRT * NCH], f32, tag="tT")  # raw chunk accums
            A8 = pP.tile([P, NRT], f32, tag="A8")  # sum m*ln(z+ns)
            B8 = pP.tile([P, NRT], f32, tag="B8")  # sum m*sim
            S8 = pP.tile([P, NRT], f32, tag="S8")  # sum m*z
            tot8 = pP.tile([P, NRT], f32, tag="tot8")
            ns8 = pP.tile([P, NRT], f32, tag="ns8")

            # ---------------- main loop over this core's row tiles ----------------
            prev = None  # (rt, c0, d0 handle) pending ln pass
            for rt in range(NRT):
                row0 = OFF + rt * P
                c0 = row0 - half  # window start column (inside chunk 0)
                lhsT_e = e0p[row0 // 512][:, row0 % 512 : row0 % 512 + P]
                m_rt = msk[:, rt, :]

                d0 = None
                for ci in range(NCH):
                    g = psP.tile([P, CH], f32, tag="g")
                    for s in range(0, CH, 512):
                        rhs = e0p[s // 512] if ci == 0 else eTc[ci][:, s : s + 512]
                        nc.tensor.matmul(
                            g[:, s : s + 512],
                            lhsT=lhsT_e,
                            rhs=rhs,
                            start=True,
                            stop=True,
                        )
                    if ci == 0:
                        d0 = pD0.tile([P, CH], bf16, tag="d0")
                        nc.scalar.activation(
                            d0,
                            g,
                            AF.Exp,
                            scale=SCALE,
                            accum_out=tT[:, rt * NCH : rt * NCH + 1],
                        )
                        # B = sum_j m*G/T over the window, straight from PSUM
                        db = pStt.tile([P, WIN], f32, tag="sttd")
                        nc.vector.scalar_tensor_tensor(
                            out=db,
                            in0=g[:, c0 : c0 + WIN],
                            scalar=SCALE,
                            in1=m_rt,
                            op0=OP.mult,
                            op1=OP.mult,
                            accum_out=B8[:, rt : rt + 1],
                        )
                        # same-label sum over the window: sum_j z*m
                        ds = pStt.tile([P, WIN], f32, tag="sttd")
                        nc.vector.scalar_tensor_tensor(
                            out=ds,
                            in0=d0[:, c0 : c0 + WIN],
                            scalar=1.0,
                            in1=m_rt,
                            op0=OP.mult,
                            op1=OP.mult,
                            accum_out=S8[:, rt : rt + 1],
                        )
                    else:
                        dd = pD.tile([P, CH], bf16, tag="dd")
                        nc.scalar.activation(
                            dd,
                            g,
                            AF.Exp,
                            scale=SCALE,
                            accum_out=tT[:, rt * NCH + ci : rt * NCH + ci + 1],
                        )

                # ns for this tile (DVE, overlaps next tile's exps)
                nc.vector.tensor_reduce(
                    tot8[:, rt : rt + 1],
                    tT[:, rt * NCH : (rt + 1) * NCH],
                    axis=mybir.AxisListType.X,
                    op=OP.add,
                )
                nc.vector.tensor_tensor(
                    ns8[:, rt : rt + 1],
                    tot8[:, rt : rt + 1],
                    S8[:, rt : rt + 1],
                    op=OP.subtract,
                )

                # ln pass for the PREVIOUS tile: its ns is ready by now, so
                # the ACT engine never stalls on the DVE reduction.
                if prev is not None:
                    prt, pc0, pd0 = prev
                    fw = pF.tile([P, WIN], bf16, tag="fw")
                    nc.scalar.activation(
                        fw,
                        pd0[:, pc0 : pc0 + WIN],
                        AF.Ln,
                        bias=ns8[:, prt : prt + 1],
                        scale=1.0,
                    )
                    da = pStt.tile([P, WIN], f32, tag="sttd")
                    nc.vector.scalar_tensor_tensor(
                        out=da,
                        in0=fw,
                        scalar=1.0,
                        in1=msk[:, prt, :],
                        op0=OP.mult,
                        op1=OP.mult,
                        accum_out=A8[:, prt : prt + 1],
                    )
                prev = (rt, c0, d0)

            # final pending ln pass
            prt, pc0, pd0 = prev
            fw = pF.tile([P, WIN], bf16, tag="fw")
            nc.scalar.activation(
                fw,
                pd0[:, pc0 : pc0 + WIN],
                AF.Ln,
                bias=ns8[:, prt : prt + 1],
                scale=1.0,
            )
            da = pStt.tile([P, WIN], f32, tag="sttd")
            nc.vector.scalar_tensor_tensor(
                out=da,
                in0=fw,
                scalar=1.0,
                in1=msk[:, prt, :],
                op0=OP.mult,
                op1=OP.mult,
                accum_out=A8[:, prt : prt + 1],
            )

            # ---------------- batched tail ----------------
            # fd = ln(ns + e^{1/T}) for all 8 tiles in one shot
            fd8 = pP.tile([P, NRT], f32, tag="fd8")
            nc.scalar.activation(fd8, ns8, AF.Ln, bias=es0, scale=1.0)
            t8 = pP.tile([P, NRT], f32, tag="t8")
            nc.vector.tensor_tensor(t8, A8, fd8, op=OP.subtract)
            nc.vector.tensor_tensor(t8, t8, B8, op=OP.subtract)
            # ship [P, 8] partials; host reduces (adds SCALE*B and divides)
            nc.sync.dma_start(out=d_out[:, :], in_=t8)

    _split_multi_waits(nc, mybir)
    return nc


def _plan(labels: np.ndarray):
    """Sort-by-label order, window geometry."""
    order = np.argsort(labels, kind="stable")
    counts = np.bincount(labels)
    max_cls = int(counts.max()) if counts.size else 1
    # per-row-tile window: 128 rows + margin >= max_cls-1 each side
    win = 512
    while win < B and (win - P) // 2 < max_cls - 1:
        win += 512
    win = min(win, 1024)  # window must fit inside sweep chunk 0
    off = max(256, (win - P) // 2 + 64)
    assert (win - P) // 2 >= max_cls - 1, "class too large"
    return order, counts, off, win


def _host_inputs(emb, lab, order, off, win):
    import ml_dtypes

    half = (win - P) // 2
    # L2-normalize rows on host (matches F.normalize with eps=1e-12)
    norm = np.linalg.norm(emb, axis=1, keepdims=True)
    e = emb / np.maximum(norm, 1e-12)
    in_maps = []
    for k in range(N_CORES):
        ck = np.roll(order, off - R * k)
        lab_r = lab[ck]
        # pre-transposed [D, B] so no on-device transposes are needed
        eT = np.ascontiguousarray(e[ck].T.astype(ml_dtypes.bfloat16))
        # per-row-tile same-label masks over each tile's window
        m = np.zeros((P, R // P, win), dtype=np.float32)
        for rt in range(R // P):
            row0 = off + rt * P
            c0 = row0 - half
            rl = lab_r[row0 : row0 + P]
            cl = lab_r[c0 : c0 + win]
            m[:, rt, :] = rl[:, None] == cl[None, :]
        in_maps.append(
            {
                "emb": eT,
                "msk": np.ascontiguousarray(
                    m.reshape(P, -1).astype(ml_dtypes.bfloat16)
                ),
            }
        )
    return in_maps


def kernel(embeddings: np.ndarray, labels: np.ndarray) -> np.ndarray:
    from concourse.bass_utils import run_bass_kernel_spmd

    emb = np.ascontiguousarray(np.asarray(embeddings, dtype=np.float32))
    lab = np.asarray(labels).astype(np.int64).ravel()
    assert emb.shape == (B, D) and lab.shape == (B,)

    order, counts, off, win = _plan(lab)
    in_maps = _host_inputs(emb, lab, order, off, win)

    nc = _build_program(win, off)
    res = run_bass_kernel_spmd(nc, in_maps, core_ids=list(range(N_CORES)))
    # device computes per-(partition, tile) sum(A - fd - B); the +1/T per
    # anchor row is constant
    loss_sum = (
        float(sum(np.asarray(r["out"], dtype=np.float64).sum() for r in res.results))
        + SCALE * B
    )

    n_c = counts[lab]
    valid = (n_c >= 2) & (n_c <= B - 1)
    valid_count = int((n_c - 1)[valid].sum())
    loss = loss_sum / valid_count if valid_count > 0 else 0.0
    return np.asarray([loss], dtype=np.float32)


# revision 15
# speedup vs baseline: 1.0256x; 1.0256x over previous
"""Supervised contrastive loss on 8 trn2 NeuronCores (Bass/Tile).

Full inputs -> full output. Sharding: rows of the (sorted-by-label,
per-core rolled) embedding matrix are split 1024/core. Each core
computes its 1024x8192 block of the similarity matrix against the full
embedding set in bf16 on the TensorEngine, reduces it to a partial
loss sum; host sums the 8 partials and divides by the (host-computed)
valid pair count.

v2: host-side normalize + transpose (kills the on-device preamble:
norms, 64 PE transposes, PSUM copies), 2048-wide exp chunks (4 PSUM
banks x 2 buffers), single exp per chunk with the ln pass reading the
positives window straight from the chunk-0 dump, ln pass software-
pipelined one row tile behind the exp sweeps, diagonal ln terms
batched into one instruction, +1/T constant folded to the host.

Key algebra: with z_ij = exp(sim_ij) and ns_i = sum_{labels differ} z_ij,
  pair_loss_ij = logaddexp(sim_ij, log ns_i) - sim_ij
              = ln(z_ij + ns_i) - sim_ij
Rows are sorted by label and rolled per-core so that all positives
(same-label columns) of each 128-row tile live in one 512-wide window
inside exp chunk 0. Same-label masks are tiny and data-dependent, so
they are precomputed host-side and DMA'd in.
"""

import math
import os
import sys

import numpy as np

for _p in ("/opt/trn_rl_repo", "/root/.axon_site/_ro/trn_rl_repo"):
    if os.path.isdir(_p) and _p not in sys.path:
        sys.path.append(_p)

B = 8192
D = 128
TEMP = 0.07
SCALE = 1.0 / TEMP
N_CORES = 8
R = B // N_CORES  # rows per core
P = 128  # partitions
CH = 2048  # exp sweep chunk width (4 psum banks, 2 buffers = all 8)
EXP_S0 = math.exp(SCALE)  # z_ii for a unit-norm row


def _split_multi_waits(nc, mybir, max_waits=1):
    """Hoist excess per-instruction sync waits onto same-engine NoOps.

    This container's walrus rejects instructions carrying more than one
    sync wait ("Too many sync wait commands"); semantics are identical
    when the preceding NoOps on the same engine perform the waits.
    """
    n_new = 0
    for func in nc.m.functions:
        for block in func.blocks:
            il = block.instructions
            i = 0
            while i < len(il):
                inst = il[i]
                si = getattr(inst, "sync_info", None)
                ow = list(si.on_wait) if (si is not None and si.on_wait) else []
                if len(ow) > max_waits:
                    keep = ow[-max_waits:]
                    hoist = ow[:-max_waits]
                    nops = []
                    for w in hoist:
                        nop = mybir.InstNoOp(
                            name=f"{inst.name}-ws{len(nops)}",
                            engine=inst.engine,
                            ins=[],
                            outs=[],
                            sync_info=mybir.SyncInfo(on_wait=[w], on_update=[]),
                        )
                        nops.append(nop)
                        n_new += 1
                    inst.sync_info = mybir.SyncInfo(
                        on_wait=keep,
                        on_update=list(si.on_update) if si.on_update else [],
                    )
                    il[i:i] = nops
                    i += len(nops)
                i += 1
    return n_new


def _build_program(WIN: int, OFF: int):
    import concourse.bass as bass
    import concourse.tile as tile
    from concourse import mybir

    f32 = mybir.dt.float32
    bf16 = mybir.dt.bfloat16
    AF = mybir.ActivationFunctionType
    OP = mybir.AluOpType

    nc = bass.Bass()
    # emb arrives pre-normalized and pre-transposed: [D, B]
    d_emb = nc.dram_tensor("emb", [D, B], bf16, kind="ExternalInput")
    d_msk = nc.dram_tensor("msk", [P, (R // P) * WIN], bf16, kind="ExternalInput")
    # per-(partition, row-tile) partial losses; host does the final reduce
    d_out = nc.dram_tensor("out", [P, R // P], f32, kind="ExternalOutput")

    NRT = R // P  # 8 row tiles owned by this core
    NCH = B // CH  # 4 exp chunks
    half = (WIN - P) // 2  # window margin each side of the 128 rows
    assert OFF - half >= 0
    assert OFF + (NRT - 1) * P - half + WIN <= CH, "window exceeds chunk 0"
    assert OFF + NRT * P <= CH, "lhsT columns exceed chunk 0"

    with tile.TileContext(nc) as tc:
        with (
            tc.tile_pool(name="emb", bufs=1) as pE,
            tc.tile_pool(name="consts", bufs=1) as pC,
            tc.tile_pool(name="parts", bufs=1) as pP,
            tc.tile_pool(name="d0", bufs=2) as pD0,
            tc.tile_pool(name="dd", bufs=2) as pD,
            tc.tile_pool(name="fw", bufs=2) as pF,
            tc.tile_pool(name="sttd", bufs=2) as pStt,
            tc.tile_pool(name="ps", bufs=2, space="PSUM") as psP,
        ):
            # ---------------- load ----------------
            # chunk 0 arrives as two 1024-col halves so the first matmuls
            # (lhsT + first rhs live in cols 0..1024) start ~2us earlier
            e0a = pE.tile([P, CH // 2], bf16, tag="e0a")
            nc.sync.dma_start(out=e0a, in_=d_emb[:, 0 : CH // 2])
            e0b = pE.tile([P, CH // 2], bf16, tag="e0b")
            nc.sync.dma_start(out=e0b, in_=d_emb[:, CH // 2 : CH])
            eTc = [None]
            for c in range(1, NCH):
                t = pE.tile([P, CH], bf16, tag=f"e{c}")
                nc.sync.dma_start(out=t, in_=d_emb[:, c * CH : (c + 1) * CH])
                eTc.append(t)
            msk = pC.tile([P, NRT, WIN], bf16, tag="msk")
            nc.sync.dma_start(
                out=msk, in_=d_msk[:, :].rearrange("p (t w) -> p t w", w=WIN)
            )

            es0 = pC.tile([P, 1], f32, tag="es0")
            nc.vector.memset(es0, EXP_S0)

            # ---------------- PE warm-up ----------------
            # HAM clock-gates the PE to 1.2GHz until it sees ~3.4us of
            # sustained busy. Burn dummy matmuls on a scratch tile during
            # the DMA wait so the main loop enters at 2.4GHz.
            wm = pC.tile([P, 512], bf16, tag="wm")
            nc.gpsimd.memset(wm, 0.0)
            gw = psP.tile([P, CH], f32, tag="g")
            for _ in range(8):
                nc.tensor.matmul(
                    gw[:, :512],
                    lhsT=wm[:, :P],
                    rhs=wm,
                    start=True,
                    stop=True,
                )

            # per-row-tile accumulators
            tT = pP.tile([P, NRT * NCH], f32, tag="tT")  # raw chunk accums
            A8 = pP.tile([P, NRT], f32, tag="A8")  # sum m*ln(z+ns)
            B8 = pP.tile([P, NRT], f32, tag="B8")  # sum m*sim
            S8 = pP.tile([P, NRT], f32, tag="S8")  # sum m*z
            tot8 = pP.tile([P, NRT], f32, tag="tot8")
            ns8 = pP.tile([P, NRT], f32, tag="ns8")

            # ---------------- main loop over this core's row tiles ----------------
            prev = None  # (rt, c0, d0 handle) pending ln pass
            H = CH // 2
            for rt in range(NRT):
                row0 = OFF + rt * P
                c0 = row0 - half  # window start column (inside chunk 0)
                if row0 + P <= H:
                    lhsT_e = e0a[:, row0 : row0 + P]
                else:
                    lhsT_e = e0b[:, row0 - H : row0 - H + P]
                m_rt = msk[:, rt, :]

                d0 = None
                for ci in range(NCH):
                    g = psP.tile([P, CH], f32, tag="g")
                    for s in range(0, CH, 512):
                        if ci == 0:
                            rhs = e0a[:, s : s + 512] if s < H else e0b[:, s - H : s - H + 512]
                        else:
                            rhs = eTc[ci][:, s : s + 512]
                        nc.tensor.matmul(
                            g[:, s : s + 512],
                            lhsT=lhsT_e,
                            rhs=rhs,
                            start=True,
                            stop=True,
                        )
                    if ci == 0:
                        d0 = pD0.tile([P, CH], bf16, tag="d0")
                        nc.scalar.activation(
                            d0,
                            g,
                            AF.Exp,
                            scale=SCALE,
                            accum_out=tT[:, rt * NCH : rt * NCH + 1],
                        )
                        # B = sum_j m*G/T over the window, straight from PSUM
                        db = pStt.tile([P, WIN], f32, tag="sttd")
                        nc.vector.scalar_tensor_tensor(
                            out=db,
                            in0=g[:, c0 : c0 + WIN],
                            scalar=SCALE,
                            in1=m_rt,
                            op0=OP.mult,
                            op1=OP.mult,
                            accum_out=B8[:, rt : rt + 1],
                        )
                        # same-label sum over the window: sum_j z*m
                        ds = pStt.tile([P, WIN], f32, tag="sttd")
                        nc.vector.scalar_tensor_tensor(
                            out=ds,
                            in0=d0[:, c0 : c0 + WIN],
                            scalar=1.0,
                            in1=m_rt,
                            op0=OP.mult,
                            op1=OP.mult,
                            accum_out=S8[:, rt : rt + 1],
                        )
                    else:
                        dd = pD.tile([P, CH], bf16, tag="dd")
                        nc.scalar.activation(
                            dd,
                            g,
                            AF.Exp,
                            scale=SCALE,
                            accum_out=tT[:, rt * NCH + ci : rt * NCH + ci + 1],
                        )

                # ns for this tile (DVE, overlaps next tile's exps)
                nc.vector.tensor_reduce(
                    tot8[:, rt : rt + 1],
                    tT[:, rt * NCH : (rt + 1) * NCH],
                    axis=mybir.AxisListType.X,
                    op=OP.add,
                )
                nc.vector.tensor_tensor(
                    ns8[:, rt : rt + 1],
                    tot8[:, rt : rt + 1],
                    S8[:, rt : rt + 1],
                    op=OP.subtract,
                )

                # ln pass for the PREVIOUS tile: its ns is ready by now, so
                # the ACT engine never stalls on the DVE reduction.
                if prev is not None:
                    prt, pc0, pd0 = prev
                    fw = pF.tile([P, WIN], bf16, tag="fw")
                    nc.scalar.activation(
                        fw,
                        pd0[:, pc0 : pc0 + WIN],
                        AF.Ln,
                        bias=ns8[:, prt : prt + 1],
                        scale=1.0,
                    )
                    da = pStt.tile([P, WIN], f32, tag="sttd")
                    nc.vector.scalar_tensor_tensor(
                        out=da,
                        in0=fw,
                        scalar=1.0,
                        in1=msk[:, prt, :],
                        op0=OP.mult,
                        op1=OP.mult,
                        accum_out=A8[:, prt : prt + 1],
                    )
                prev = (rt, c0, d0)

            # final pending ln pass
            prt, pc0, pd0 = prev
            fw = pF.tile([P, WIN], bf16, tag="fw")
            nc.scalar.activation(
                fw,
                pd0[:, pc0 : pc0 + WIN],
                AF.Ln,
                bias=ns8[:, prt : prt + 1],
                scale=1.0,
            )
            da = pStt.tile([P, WIN], f32, tag="sttd")
            nc.vector.scalar_tensor_tensor(
                out=da,
                in0=fw,
                scalar=1.0,
                in1=msk[:, prt, :],
                op0=OP.mult,
                op1=OP.mult,
                accum_out=A8[:, prt : prt + 1],
            )

            # ---------------- batched tail ----------------
            # fd = ln(ns + e^{1/T}) for all 8 tiles in one shot
            fd8 = pP.tile([P, NRT], f32, tag="fd8")
            nc.scalar.activation(fd8, ns8, AF.Ln, bias=es0, scale=1.0)
            t8 = pP.tile([P, NRT], f32, tag="t8")
            nc.vector.tensor_tensor(t8, A8, fd8, op=OP.subtract)
            nc.vector.tensor_tensor(t8, t8, B8, op=OP.subtract)
            # ship [P, 8] partials; host reduces (adds SCALE*B and divides)
            nc.sync.dma_start(out=d_out[:, :], in_=t8)

    _split_multi_waits(nc, mybir)
    return nc


def _plan(labels: np.ndarray):
    """Sort-by-label order, window geometry."""
    order = np.argsort(labels, kind="stable")
    counts = np.bincount(labels)
    max_cls = int(counts.max()) if counts.size else 1
    # per-row-tile window: 128 rows + margin >= max_cls-1 each side
    win = 512
    while win < B and (win - P) // 2 < max_cls - 1:
        win += 512
    win = min(win, 1024)  # window must fit inside sweep chunk 0
    off = max(256, (win - P) // 2 + 64)
    assert (win - P) // 2 >= max_cls - 1, "class too large"
    return order, counts, off, win


def _host_inputs(emb, lab, order, off, win):
    import ml_dtypes

    half = (win - P) // 2
    # L2-normalize rows on host (matches F.normalize with eps=1e-12)
    norm = np.linalg.norm(emb, axis=1, keepdims=True)
    e = emb / np.maximum(norm, 1e-12)
    in_maps = []
    for k in range(N_CORES):
        ck = np.roll(order, off - R * k)
        lab_r = lab[ck]
        # pre-transposed [D, B] so no on-device transposes are needed
        eT = np.ascontiguousarray(e[ck].T.astype(ml_dtypes.bfloat16))
        # per-row-tile same-label masks over each tile's window
        m = np.zeros((P, R // P, win), dtype=np.float32)
        for rt in range(R // P):
            row0 = off + rt * P
            c0 = row0 - half
            rl = lab_r[row0 : row0 + P]
            cl = lab_r[c0 : c0 + win]
            m[:, rt, :] = rl[:, None] == cl[None, :]
        in_maps.append(
            {
                "emb": eT,
                "msk": np.ascontiguousarray(
                    m.reshape(P, -1).astype(ml_dtypes.bfloat16)
                ),
            }
        )
    return in_maps


def kernel(embeddings: np.ndarray, labels: np.ndarray) -> np.ndarray:
    from concourse.bass_utils import run_bass_kernel_spmd

    emb = np.ascontiguousarray(np.asarray(embeddings, dtype=np.float32))
    lab = np.asarray(labels).astype(np.int64).ravel()
    assert emb.shape == (B, D) and lab.shape == (B,)

    order, counts, off, win = _plan(lab)
    in_maps = _host_inputs(emb, lab, order, off, win)

    nc = _build_program(win, off)
    res = run_bass_kernel_spmd(nc, in_maps, core_ids=list(range(N_CORES)))
    # device computes per-(partition, tile) sum(A - fd - B); the +1/T per
    # anchor row is constant
    loss_sum = (
        float(sum(np.asarray(r["out"], dtype=np.float64).sum() for r in res.results))
        + SCALE * B
    )

    n_c = counts[lab]
    valid = (n_c >= 2) & (n_c <= B - 1)
    valid_count = int((n_c - 1)[valid].sum())
    loss = loss_sum / valid_count if valid_count > 0 else 0.0
    return np.asarray([loss], dtype=np.float32)


# revision 17
# speedup vs baseline: 1.0565x; 1.0302x over previous
"""Supervised contrastive loss on 8 trn2 NeuronCores (Bass/Tile).

Full inputs -> full output. Sharding: rows of the (sorted-by-label,
per-core rolled) embedding matrix are split 1024/core. Each core
computes its 1024x8192 block of the similarity matrix against the full
embedding set in bf16 on the TensorEngine, reduces it to a partial
loss sum; host sums the 8 partials and divides by the (host-computed)
valid pair count.

v2: host-side normalize + transpose (kills the on-device preamble:
norms, 64 PE transposes, PSUM copies), 2048-wide exp chunks (4 PSUM
banks x 2 buffers), single exp per chunk with the ln pass reading the
positives window straight from the chunk-0 dump, ln pass software-
pipelined one row tile behind the exp sweeps, diagonal ln terms
batched into one instruction, +1/T constant folded to the host.

Key algebra: with z_ij = exp(sim_ij) and ns_i = sum_{labels differ} z_ij,
  pair_loss_ij = logaddexp(sim_ij, log ns_i) - sim_ij
              = ln(z_ij + ns_i) - sim_ij
Rows are sorted by label and rolled per-core so that all positives
(same-label columns) of each 128-row tile live in one 512-wide window
inside exp chunk 0. Same-label masks are tiny and data-dependent, so
they are precomputed host-side and DMA'd in.
"""

import math
import os
import sys

import numpy as np

for _p in ("/opt/trn_rl_repo", "/root/.axon_site/_ro/trn_rl_repo"):
    if os.path.isdir(_p) and _p not in sys.path:
        sys.path.append(_p)

B = 8192
D = 128
TEMP = 0.07
SCALE = 1.0 / TEMP
N_CORES = 8
R = B // N_CORES  # rows per core
P = 128  # partitions
CH = 2048  # exp sweep chunk width (4 psum banks, 2 buffers = all 8)
EXP_S0 = math.exp(SCALE)  # z_ii for a unit-norm row


def _split_multi_waits(nc, mybir, max_waits=1):
    """Hoist excess per-instruction sync waits onto same-engine NoOps.

    This container's walrus rejects instructions carrying more than one
    sync wait ("Too many sync wait commands"); semantics are identical
    when the preceding NoOps on the same engine perform the waits.
    """
    n_new = 0
    for func in nc.m.functions:
        for block in func.blocks:
            il = block.instructions
            i = 0
            while i < len(il):
                inst = il[i]
                si = getattr(inst, "sync_info", None)
                ow = list(si.on_wait) if (si is not None and si.on_wait) else []
                if len(ow) > max_waits:
                    keep = ow[-max_waits:]
                    hoist = ow[:-max_waits]
                    nops = []
                    for w in hoist:
                        nop = mybir.InstNoOp(
                            name=f"{inst.name}-ws{len(nops)}",
                            engine=inst.engine,
                            ins=[],
                            outs=[],
                            sync_info=mybir.SyncInfo(on_wait=[w], on_update=[]),
                        )
                        nops.append(nop)
                        n_new += 1
                    inst.sync_info = mybir.SyncInfo(
                        on_wait=keep,
                        on_update=list(si.on_update) if si.on_update else [],
                    )
                    il[i:i] = nops
                    i += len(nops)
                i += 1
    return n_new


def _build_program(WIN: int, OFF: int):
    import concourse.bass as bass
    import concourse.tile as tile
    from concourse import mybir

    f32 = mybir.dt.float32
    bf16 = mybir.dt.bfloat16
    AF = mybir.ActivationFunctionType
    OP = mybir.AluOpType

    nc = bass.Bass()
    # emb arrives pre-normalized and pre-transposed: [D, B]
    d_emb = nc.dram_tensor("emb", [D, B], bf16, kind="ExternalInput")
    d_msk = nc.dram_tensor("msk", [P, (R // P) * WIN], bf16, kind="ExternalInput")
    # per-(partition, row-tile) partial losses; host does the final reduce
    d_out = nc.dram_tensor("out", [P, R // P], f32, kind="ExternalOutput")

    NRT = R // P  # 8 row tiles owned by this core
    NCH = B // CH  # 4 exp chunks
    half = (WIN - P) // 2  # window margin each side of the 128 rows
    assert OFF - half >= 0
    assert OFF + (NRT - 1) * P - half + WIN <= CH, "window exceeds chunk 0"
    assert OFF + NRT * P <= CH, "lhsT columns exceed chunk 0"

    with tile.TileContext(nc) as tc:
        with (
            tc.tile_pool(name="emb", bufs=1) as pE,
            tc.tile_pool(name="consts", bufs=1) as pC,
            tc.tile_pool(name="parts", bufs=1) as pP,
            tc.tile_pool(name="d0", bufs=2) as pD0,
            tc.tile_pool(name="dd", bufs=2) as pD,
            tc.tile_pool(name="di", bufs=2) as pDI,
            tc.tile_pool(name="fw", bufs=2) as pF,
            tc.tile_pool(name="sttd", bufs=2) as pStt,
            tc.tile_pool(name="ps", bufs=2, space="PSUM") as psP,
        ):
            # ---------------- load ----------------
            # chunk 0 arrives as two 1024-col halves so the first matmuls
            # (lhsT + first rhs live in cols 0..1024) start ~2us earlier
            e0a = pE.tile([P, CH // 2], bf16, tag="e0a")
            nc.sync.dma_start(out=e0a, in_=d_emb[:, 0 : CH // 2])
            e0b = pE.tile([P, CH // 2], bf16, tag="e0b")
            nc.sync.dma_start(out=e0b, in_=d_emb[:, CH // 2 : CH])
            eTc = [None]
            for c in range(1, NCH):
                t = pE.tile([P, CH], bf16, tag=f"e{c}")
                nc.sync.dma_start(out=t, in_=d_emb[:, c * CH : (c + 1) * CH])
                eTc.append(t)
            msk = pC.tile([P, NRT, WIN], bf16, tag="msk")
            nc.sync.dma_start(
                out=msk, in_=d_msk[:, :].rearrange("p (t w) -> p t w", w=WIN)
            )

            es0 = pC.tile([P, 1], f32, tag="es0")
            nc.vector.memset(es0, EXP_S0)

            # ---------------- PE warm-up ----------------
            # HAM clock-gates the PE to 1.2GHz until it sees ~3.4us of
            # sustained busy. Burn dummy matmuls on a scratch tile during
            # the DMA wait so the main loop enters at 2.4GHz.
            wm = pC.tile([P, 512], bf16, tag="wm")
            nc.gpsimd.memset(wm, 0.0)
            gw = psP.tile([P, CH], f32, tag="g")
            for _ in range(8):
                nc.tensor.matmul(
                    gw[:, :512],
                    lhsT=wm[:, :P],
                    rhs=wm,
                    start=True,
                    stop=True,
                )

            # per-row-tile accumulators
            tT = pP.tile([P, NRT * NCH], f32, tag="tT")  # raw chunk accums
            A8 = pP.tile([P, NRT], f32, tag="A8")  # sum m*ln(z+ns)
            B8 = pP.tile([P, NRT], f32, tag="B8")  # sum m*sim
            S8 = pP.tile([P, NRT], f32, tag="S8")  # sum m*z
            tot8 = pP.tile([P, NRT], f32, tag="tot8")
            ns8 = pP.tile([P, NRT], f32, tag="ns8")

            # ---------------- main loop over this core's row tiles ----------------
            prev = None  # (rt, c0, d0 handle) pending ln pass
            H = CH // 2
            for rt in range(NRT):
                row0 = OFF + rt * P
                c0 = row0 - half  # window start column (inside chunk 0)
                if row0 + P <= H:
                    lhsT_e = e0a[:, row0 : row0 + P]
                else:
                    lhsT_e = e0b[:, row0 - H : row0 - H + P]
                m_rt = msk[:, rt, :]

                d0 = None
                for ci in range(NCH):
                    g = psP.tile([P, CH], f32, tag="g")
                    for s in range(0, CH, 512):
                        if ci == 0:
                            rhs = e0a[:, s : s + 512] if s < H else e0b[:, s - H : s - H + 512]
                        else:
                            rhs = eTc[ci][:, s : s + 512]
                        nc.tensor.matmul(
                            g[:, s : s + 512],
                            lhsT=lhsT_e,
                            rhs=rhs,
                            start=True,
                            stop=True,
                        )
                    if ci == 0:
                        d0 = pD0.tile([P, CH], bf16, tag="d0")
                        nc.scalar.activation(
                            d0,
                            g,
                            AF.Exp,
                            scale=SCALE,
                            accum_out=tT[:, rt * NCH : rt * NCH + 1],
                        )
                        # B = sum_j m*G/T over the window, straight from PSUM
                        db = pStt.tile([P, WIN], f32, tag="sttd")
                        nc.vector.scalar_tensor_tensor(
                            out=db,
                            in0=g[:, c0 : c0 + WIN],
                            scalar=SCALE,
                            in1=m_rt,
                            op0=OP.mult,
                            op1=OP.mult,
                            accum_out=B8[:, rt : rt + 1],
                        )
                        # same-label sum over the window: sum_j z*m
                        ds = pStt.tile([P, WIN], f32, tag="sttd")
                        nc.vector.scalar_tensor_tensor(
                            out=ds,
                            in0=d0[:, c0 : c0 + WIN],
                            scalar=1.0,
                            in1=m_rt,
                            op0=OP.mult,
                            op1=OP.mult,
                            accum_out=S8[:, rt : rt + 1],
                        )
                    elif ci == NCH - 1:
                        # Schraudolph fast-exp on the (otherwise idle) DVE:
                        # exp(SCALE*g) ~= bitcast_f32(int32(g*A + B)); this
                        # chunk's z values are only ever summed, and the
                        # ~0.5% systematic error on 1/4 of ns shifts the
                        # loss by <2e-4 (tolerance 2e-2).
                        dI = pDI.tile([P, CH], mybir.dt.int32, tag="di")
                        nc.vector.tensor_scalar(
                            out=dI,
                            in0=g,
                            scalar1=SCALE * 12102203.161561485,
                            scalar2=1064986316.0,
                            op0=OP.mult,
                            op1=OP.add,
                        )
                        nc.vector.tensor_reduce(
                            tT[:, rt * NCH + ci : rt * NCH + ci + 1],
                            dI[:, :].bitcast(f32),
                            axis=mybir.AxisListType.X,
                            op=OP.add,
                        )
                    else:
                        dd = pD.tile([P, CH], bf16, tag="dd")
                        nc.scalar.activation(
                            dd,
                            g,
                            AF.Exp,
                            scale=SCALE,
                            accum_out=tT[:, rt * NCH + ci : rt * NCH + ci + 1],
                        )

                # ns for this tile (DVE, overlaps next tile's exps)
                nc.vector.tensor_reduce(
                    tot8[:, rt : rt + 1],
                    tT[:, rt * NCH : (rt + 1) * NCH],
                    axis=mybir.AxisListType.X,
                    op=OP.add,
                )
                nc.vector.tensor_tensor(
                    ns8[:, rt : rt + 1],
                    tot8[:, rt : rt + 1],
                    S8[:, rt : rt + 1],
                    op=OP.subtract,
                )

                # ln pass for the PREVIOUS tile: its ns is ready by now, so
                # the ACT engine never stalls on the DVE reduction.
                if prev is not None:
                    prt, pc0, pd0 = prev
                    fw = pF.tile([P, WIN], bf16, tag="fw")
                    nc.scalar.activation(
                        fw,
                        pd0[:, pc0 : pc0 + WIN],
                        AF.Ln,
                        bias=ns8[:, prt : prt + 1],
                        scale=1.0,
                    )
                    da = pStt.tile([P, WIN], f32, tag="sttd")
                    nc.vector.scalar_tensor_tensor(
                        out=da,
                        in0=fw,
                        scalar=1.0,
                        in1=msk[:, prt, :],
                        op0=OP.mult,
                        op1=OP.mult,
                        accum_out=A8[:, prt : prt + 1],
                    )
                prev = (rt, c0, d0)

            # final pending ln pass
            prt, pc0, pd0 = prev
            fw = pF.tile([P, WIN], bf16, tag="fw")
            nc.scalar.activation(
                fw,
                pd0[:, pc0 : pc0 + WIN],
                AF.Ln,
                bias=ns8[:, prt : prt + 1],
                scale=1.0,
            )
            da = pStt.tile([P, WIN], f32, tag="sttd")
            nc.vector.scalar_tensor_tensor(
                out=da,
                in0=fw,
                scalar=1.0,
                in1=msk[:, prt, :],
                op0=OP.mult,
                op1=OP.mult,
                accum_out=A8[:, prt : prt + 1],
            )

            # ---------------- batched tail ----------------
            # fd = ln(ns + e^{1/T}) for all 8 tiles in one shot
            fd8 = pP.tile([P, NRT], f32, tag="fd8")
            nc.scalar.activation(fd8, ns8, AF.Ln, bias=es0, scale=1.0)
            t8 = pP.tile([P, NRT], f32, tag="t8")
            nc.vector.tensor_tensor(t8, A8, fd8, op=OP.subtract)
            nc.vector.tensor_tensor(t8, t8, B8, op=OP.subtract)
            # ship [P, 8] partials; host reduces (adds SCALE*B and divides)
            nc.sync.dma_start(out=d_out[:, :], in_=t8)

    _split_multi_waits(nc, mybir)
    return nc


def _plan(labels: np.ndarray):
    """Sort-by-label order, window geometry."""
    order = np.argsort(labels, kind="stable")
    counts = np.bincount(labels)
    max_cls = int(counts.max()) if counts.size else 1
    # per-row-tile window: 128 rows + margin >= max_cls-1 each side
    win = 512
    while win < B and (win - P) // 2 < max_cls - 1:
        win += 512
    win = min(win, 1024)  # window must fit inside sweep chunk 0
    off = max(256, (win - P) // 2 + 64)
    assert (win - P) // 2 >= max_cls - 1, "class too large"
    return order, counts, off, win


def _host_inputs(emb, lab, order, off, win):
    import ml_dtypes

    half = (win - P) // 2
    # L2-normalize rows on host (matches F.normalize with eps=1e-12)
    norm = np.linalg.norm(emb, axis=1, keepdims=True)
    e = emb / np.maximum(norm, 1e-12)
    in_maps = []
    for k in range(N_CORES):
        ck = np.roll(order, off - R * k)
        lab_r = lab[ck]
        # pre-transposed [D, B] so no on-device transposes are needed
        eT = np.ascontiguousarray(e[ck].T.astype(ml_dtypes.bfloat16))
        # per-row-tile same-label masks over each tile's window
        m = np.zeros((P, R // P, win), dtype=np.float32)
        for rt in range(R // P):
            row0 = off + rt * P
            c0 = row0 - half
            rl = lab_r[row0 : row0 + P]
            cl = lab_r[c0 : c0 + win]
            m[:, rt, :] = rl[:, None] == cl[None, :]
        in_maps.append(
            {
                "emb": eT,
                "msk": np.ascontiguousarray(
                    m.reshape(P, -1).astype(ml_dtypes.bfloat16)
                ),
            }
        )
    return in_maps


def kernel(embeddings: np.ndarray, labels: np.ndarray) -> np.ndarray:
    from concourse.bass_utils import run_bass_kernel_spmd

    emb = np.ascontiguousarray(np.asarray(embeddings, dtype=np.float32))
    lab = np.asarray(labels).astype(np.int64).ravel()
    assert emb.shape == (B, D) and lab.shape == (B,)

    order, counts, off, win = _plan(lab)
    in_maps = _host_inputs(emb, lab, order, off, win)

    nc = _build_program(win, off)
    res = run_bass_kernel_spmd(nc, in_maps, core_ids=list(range(N_CORES)))
    # device computes per-(partition, tile) sum(A - fd - B); the +1/T per
    # anchor row is constant
    loss_sum = (
        float(sum(np.asarray(r["out"], dtype=np.float64).sum() for r in res.results))
        + SCALE * B
    )

    n_c = counts[lab]
    valid = (n_c >= 2) & (n_c <= B - 1)
    valid_count = int((n_c - 1)[valid].sum())
    loss = loss_sum / valid_count if valid_count > 0 else 0.0
    return np.asarray([loss], dtype=np.float32)
